# revision 64
# baseline (speedup 1.0000x reference)
"""BPTransformer Trainium2 kernel: 8-core SPMD bass implementation.

Sharding: detector-split backprojection (16 dets/NC, all 4 batches,
batch+shift packed on partitions), ReduceScatter (doubled slots) so NC i
receives batch i%4's full backprojected image, then a per-NC transformer
on its batch. Host assembles outputs from cores 0-3.
"""

import numpy as np

import concourse.bass as bass
import concourse.mybir as mybir
import concourse.tile as tile
from concourse import bacc, bass_utils

# problem shapes (hardcoded per contract)
B, NDET, NT, NY, NX = 4, 128, 2048, 256, 256
D, L, NH, PS = 256, 6, 8, 16
HP, WP = NY // PS, NX // PS
N = HP * WP          # 256 tokens
HD = D // NH         # 32
NPIX = NY * NX       # 65536
N_CORES = 8
N_PASS = 2           # dets per q7 core
N_CHUNK = 16
CHUNK = NPIX // N_CHUNK         # 4096
F32 = mybir.dt.float32
BF16 = mybir.dt.bfloat16
I16 = mybir.dt.int16
AT = mybir.AluOpType
AF = mybir.ActivationFunctionType


# bias-column layout in the packed [128, NBCOL] bias tensor
# qkv biases use 12 columns of 64 rows each; the rest use 128 rows.
def _bcol(l, kind, m):
    return l * 24 + {"qkv": 0, "bo": 12, "b1": 14, "b2": 22}[kind] + m


BCOL_PROJ = L * 24          # 144, 2 cols
BCOL_CONVW = BCOL_PROJ + 2  # 146, 9 cols
BCOL_CONVB = BCOL_CONVW + 9  # 155
NBCOL = 156


def build_nc():
    nc = bacc.Bacc("TRN2", target_bir_lowering=False, debug=False,
                   num_devices=N_CORES)

    # ---- per-core inputs ----
    tbl_d = nc.dram_tensor("tbl", [128, N_PASS * NT], F32, kind="ExternalInput")
    idx_d = nc.dram_tensor("idx", [N_PASS, N_CHUNK, 128, CHUNK // 16], I16,
                           kind="ExternalInput")
    wts_d = nc.dram_tensor("wts", [N_PASS, N_CHUNK, 128, CHUNK], F32,
                           kind="ExternalInput")
    sel_d = nc.dram_tensor("sel", [128, 4], F32, kind="ExternalInput")
    # ---- shared transformer weights (matmul operands in bf16) ----
    wqkvT_d = nc.dram_tensor("wqkvT", [L, 2, 128, 3 * D], BF16, kind="ExternalInput")
    woT_d = nc.dram_tensor("woT", [L, 2, 128, D], BF16, kind="ExternalInput")
    w1T_d = nc.dram_tensor("w1T", [L, 2, 128, 4 * D], BF16, kind="ExternalInput")
    w2T_d = nc.dram_tensor("w2T", [L, 8, 128, D], BF16, kind="ExternalInput")
    pwT_d = nc.dram_tensor("pwT", [2, 128, D], BF16, kind="ExternalInput")
    projT_d = nc.dram_tensor("projT", [2, 128, PS * PS], BF16, kind="ExternalInput")
    posb_d = nc.dram_tensor("posb", [2, 128, D], F32, kind="ExternalInput")
    lnrep_d = nc.dram_tensor("lnrep", [L, 4, 128, D], F32, kind="ExternalInput")
    bias_d = nc.dram_tensor("biases", [128, NBCOL], F32, kind="ExternalInput")
    shifts_d = nc.dram_tensor("shifts", [128, 4 * 128], F32, kind="ExternalInput")
    bvrep_d = nc.dram_tensor("bvrep", [L, 128, D], F32, kind="ExternalInput")
    ident_d = nc.dram_tensor("identm", [128, 128], F32, kind="ExternalInput")
    identb_d = nc.dram_tensor("identb", [128, 128], BF16, kind="ExternalInput")
    hones_d = nc.dram_tensor("hones", [NH, 128, NH], BF16, kind="ExternalInput")
    expand_d = nc.dram_tensor("expand2", [2, 8, 128], BF16, kind="ExternalInput")

    # ---- outputs ----
    img_o = nc.dram_tensor("img", [NPIX], F32, kind="ExternalOutput")
    bp_o = nc.dram_tensor("bp", [NPIX], F32, kind="ExternalOutput")

    NQ = 4  # reduce-scatter quarters (first 3 overlap the BP phase)
    with tile.TileContext(nc) as tc:
        with tc.tile_pool(name="dram", bufs=1, space="DRAM") as dram_pool:
            rs_inq = [dram_pool.tile([N_CORES, NPIX // NQ], F32,
                                     name=f"rs_in{q}") for q in range(NQ)]
            rs_outq = [dram_pool.tile([NPIX // NQ], F32, name=f"rs_out{q}")
                       for q in range(NQ)]
            imgbuf = dram_pool.tile([NPIX], F32)

            # ================= backprojection =================
            with (
                tc.tile_pool(name="tbl", bufs=1) as tbl_pool,
                tc.tile_pool(name="selp", bufs=1) as sel_pool,
                tc.tile_pool(name="gidx", bufs=2) as idx_pool,
                tc.tile_pool(name="gout", bufs=2) as g_pool,
                tc.tile_pool(name="gwts", bufs=2) as w_pool,
                tc.tile_pool(name="gacc", bufs=2) as acc_pool,
                tc.tile_pool(name="gtmp", bufs=1) as tmp_pool,
                tc.tile_pool(name="bpsb", bufs=1) as bp_pool,
                tc.tile_pool(name="bpps", bufs=4, space="PSUM") as bp_psum,
            ):
                tbl = tbl_pool.tile([128, N_PASS * NT], F32)
                nc.sync.dma_start(tbl[:], tbl_d.ap())
                sel = sel_pool.tile([128, 4], F32)
                nc.sync.dma_start(sel[:], sel_d.ap())

                for c in range(N_CHUNK):
                    acc = acc_pool.tile([128, CHUNK], F32, tag="acc", name="acc")
                    for p in range(N_PASS):
                        it = idx_pool.tile([128, CHUNK // 16], I16, tag="it", name="it")
                        nc.sync.dma_start(it[:], idx_d.ap()[p, c])
                        wt = w_pool.tile([128, CHUNK], F32, tag="wt", name="wt")
                        nc.sync.dma_start(wt[:], wts_d.ap()[p, c])
                        g = g_pool.tile([128, CHUNK], F32, tag="g", name="g")
                        nc.gpsimd.ap_gather(
                            g[:], tbl[:, p * NT:(p + 1) * NT], it[:],
                            channels=128, num_elems=NT, d=1, num_idxs=CHUNK)
                        if p == 0:
                            nc.vector.tensor_tensor(acc[:], g[:], wt[:], AT.mult)
                        else:
                            tmp = tmp_pool.tile([128, CHUNK], F32, tag="tmp", name="tmp")
                            nc.vector.tensor_tensor(tmp[:], g[:], wt[:], AT.mult)
                            nc.vector.tensor_tensor(acc[:], acc[:], tmp[:], AT.add)
                    # reduce over partitions (dets x shifts) into 4 batch rows
                    bp_sb = bp_pool.tile([4, CHUNK], F32, tag="bpsb", name="bpsb")
                    for blk in range(CHUNK // 512):
                        ps = bp_psum.tile([4, 512], F32, tag="bps", name="bps")
                        nc.tensor.matmul(ps[:], sel[:],
                                         acc[:, blk * 512:(blk + 1) * 512],
                                         start=True, stop=True)
                        nc.scalar.copy(bp_sb[:, blk * 512:(blk + 1) * 512], ps[:])
                    # doubled slots: slot j carries batch j%4
                    cpq = N_CHUNK // NQ
                    rsd = rs_inq[c // cpq]
                    cc = c % cpq
                    nc.sync.dma_start(
                        rsd[0:4, cc * CHUNK:(cc + 1) * CHUNK], bp_sb[:])
                    nc.sync.dma_start(
                        rsd[4:8, cc * CHUNK:(cc + 1) * CHUNK], bp_sb[:])
                    # fire each quarter's reduce-scatter as soon as its
                    # chunks are out, so 3 of 4 overlap remaining BP work
                    if cc == cpq - 1:
                        q = c // cpq
                        nc.gpsimd.collective_compute(
                            "ReduceScatter", AT.add,
                            replica_groups=[list(range(N_CORES))],
                            ins=[rs_inq[q][:].opt()],
                            outs=[rs_outq[q][:].opt()])
                        nc.sync.dma_start(
                            bp_o.ap()[q * (NPIX // NQ):(q + 1) * (NPIX // NQ)],
                            rs_outq[q][:])

            # ================= transformer =================
            _transformer(nc, tc, rs_outq, imgbuf,
                         wqkvT_d, woT_d, w1T_d, w2T_d, pwT_d, projT_d,
                         posb_d, lnrep_d, bias_d, shifts_d, bvrep_d, ident_d,
                         identb_d, hones_d, expand_d, img_o)

    nc.compile()
    return nc


def _transpose_256(nc, psum_tr, dst_pool, src_tiles, ident, tag="tr", dt=F32):
    """src: 2 tiles/APs [128, 256] (partition-chunked 256x256 matrix).
    returns 2 tiles [128, 256]: the transposed matrix, partition-chunked.
    `ident` dtype must match src dtype; dst tiles take dtype `dt`."""
    dst = [dst_pool.tile([128, 256], dt, tag=f"{tag}{j}", name=f"{tag}{j}") for j in range(2)]
    src_dt = src_tiles[0].dtype if hasattr(src_tiles[0], "dtype") else F32
    for ci in range(2):          # src partition chunk
        for fj in range(2):      # src free chunk
            ps = psum_tr.tile([128, 128], src_dt, tag="trp", name="trp")
            nc.tensor.transpose(ps[:], src_tiles[ci][:, fj * 128:(fj + 1) * 128],
                                ident[:])
            nc.vector.tensor_copy(dst[fj][:, ci * 128:(ci + 1) * 128], ps[:])
    return dst


def _layernorm(nc, sc_pool, out_pool, x_tiles):
    """row-layout LN over free dim D (affine gamma/beta folded into the
    downstream matmul weights/biases host-side). x_tiles: 2 x [128, D]."""
    out = []
    for ct in range(2):
        x = x_tiles[ct]
        s = sc_pool.tile([128, 1], F32, tag="s", name="s")
        nc.vector.reduce_sum(s[:], x[:], mybir.AxisListType.X)
        nm = sc_pool.tile([128, 1], F32, tag="nm", name="nm")
        nc.vector.tensor_scalar(nm[:], s[:], -1.0 / D, None, AT.mult)
        xc = out_pool.tile([128, D], F32, tag="xc", name="xc")
        nc.vector.tensor_scalar(xc[:], x[:], nm[:], None, AT.add)
        sq = out_pool.tile([128, D], F32, tag="sq", name="sq")
        nc.vector.tensor_tensor(sq[:], xc[:], xc[:], AT.mult)
        vs = sc_pool.tile([128, 1], F32, tag="vs", name="vs")
        nc.vector.reduce_sum(vs[:], sq[:], mybir.AxisListType.X)
        vs2 = sc_pool.tile([128, 1], F32, tag="vs2", name="vs2")
        nc.vector.tensor_scalar(vs2[:], vs[:], 1.0 / D, 1e-5, AT.mult, AT.add)
        std = sc_pool.tile([128, 1], F32, tag="std", name="std")
        nc.scalar.activation(std[:], vs2[:], AF.Sqrt)
        inv = sc_pool.tile([128, 1], F32, tag="inv", name="inv")
        nc.vector.reciprocal(inv[:], std[:])
        y = out_pool.tile([128, D], F32, tag="lny", name="lny")
        nc.vector.tensor_scalar(y[:], xc[:], inv[:], None, AT.mult)
        out.append(y)
    return out


def _transformer(nc, tc, rs_outq, imgbuf, wqkvT_d, woT_d, w1T_d, w2T_d,
                 pwT_d, projT_d, posb_d, lnrep_d, bias_d, shifts_d, bvrep_d,
                 ident_d, identb_d, hones_d, expand_d, img_o):
    with (
        tc.tile_pool(name="ident", bufs=1) as id_pool,
        tc.tile_pool(name="bias", bufs=1) as bias_pool,
        tc.tile_pool(name="zst", bufs=1) as z_pool,          # persistent z
        tc.tile_pool(name="lnv", bufs=2) as ln_pool,
        tc.tile_pool(name="wq", bufs=2) as wq_pool,
        tc.tile_pool(name="act", bufs=2) as act_pool,        # transient acts
        tc.tile_pool(name="sc", bufs=8) as sc_pool,          # [128,1] scalars
        tc.tile_pool(name="attn", bufs=2) as attn_pool,
        tc.tile_pool(name="pst", bufs=2, space="PSUM") as psum_tr,
        tc.tile_pool(name="psm", bufs=2, space="PSUM") as psum_mm,
        tc.tile_pool(name="pss", bufs=2, space="PSUM") as psum_sc,
        tc.tile_pool(name="psd", bufs=1, space="PSUM") as psum_den,
    ):
        ident = id_pool.tile([128, 128], F32, name="ident")
        nc.sync.dma_start(ident[:], ident_d.ap())
        identb = id_pool.tile([128, 128], BF16, name="identb")
        nc.sync.dma_start(identb[:], identb_d.ap())
        hones = [id_pool.tile([128, NH], BF16, tag=f"ho{h}", name=f"ho{h}")
                 for h in range(NH)]
        for h in range(NH):
            nc.sync.dma_start(hones[h][:], hones_d.ap()[h])
        expt = [id_pool.tile([8, 128], BF16, tag=f"expt{j}", name=f"expt{j}")
                for j in range(2)]
        for j in range(2):
            nc.sync.dma_start(expt[j][:], expand_d.ap()[j])
        biases = bias_pool.tile([128, NBCOL], F32, name="biases")
        nc.sync.dma_start(biases[:], bias_d.ap())

        bp_res = [q[:].rearrange("(a c d e) -> c a d e", a=4, c=16, d=16, e=16)
                  for q in rs_outq]

        # ---- patch extract: X [tok, (c,e)] row layout ----
        X = [act_pool.tile([128, 256], F32, tag=f"X{j}", name=f"X{j}") for j in range(2)]
        for ch in range(2):
            for c in range(16):
                nc.sync.dma_start(X[ch][0:64, c * 16:(c + 1) * 16],
                                  bp_res[2 * ch][c])
                nc.sync.dma_start(X[ch][64:128, c * 16:(c + 1) * 16],
                                  bp_res[2 * ch + 1][c])
        XT = _transpose_256(nc, psum_tr, act_pool, X, ident, tag="XT", dt=BF16)
        # z0 [tok, D] = X @ pw.T : lhsT = XT chunks, rhs = pwT
        pw = [wq_pool.tile([128, D], BF16, tag=f"pw{k}", name=f"pw{k}") for k in range(2)]
        for k in range(2):
            nc.sync.dma_start(pw[k][:], pwT_d.ap()[k])
        posb = [wq_pool.tile([128, D], F32, tag=f"posb{j}", name=f"posb{j}") for j in range(2)]
        for ch in range(2):
            nc.sync.dma_start(posb[ch][:], posb_d.ap()[ch])
        z = [z_pool.tile([128, D], F32, tag=f"z{ch}", name=f"z{ch}") for ch in range(2)]
        for m in range(2):  # token chunk
            ps = psum_mm.tile([128, D], F32, tag="mmp", name="mmp")
            for k in range(2):  # ce chunk
                nc.tensor.matmul(ps[:], XT[k][:, m * 128:(m + 1) * 128],
                                 pw[k][:], start=(k == 0), stop=(k == 1))
            nc.vector.tensor_tensor(z[m][:], ps[:], posb[m][:], AT.add)

        scale = 1.0 / float(np.sqrt(HD))
        for l in range(L):
            # ---- LN1 ----
            y = _layernorm(nc, sc_pool, act_pool, z)
            yT = _transpose_256(nc, psum_tr, act_pool, y, ident, tag="yT",
                                dt=BF16)

            # ---- qkv [feat, tok] ----
            wq = [wq_pool.tile([128, 3 * D], BF16, tag=f"wqkv{k}", name=f"wqkv{k}") for k in range(2)]
            for k in range(2):
                nc.sync.dma_start(wq[k][:], wqkvT_d.ap()[l, k])
            qkv = [act_pool.tile([64, 256], BF16, tag=f"qkv{m}", name=f"qkv{m}") for m in range(8)]
            for m in range(8):
                ps = psum_mm.tile([64, 256], F32, tag="mmp", name="mmp")
                for k in range(2):
                    nc.tensor.matmul(ps[:], wq[k][:, m * 64:(m + 1) * 64],
                                     yT[k][:], start=(k == 0), stop=(k == 1))
                bc = _bcol(l, "qkv", m)
                nc.vector.tensor_scalar(qkv[m][:], ps[:],
                                        biases[0:64, bc:bc + 1], None, AT.add)
            # vT [tok, v-feat] computed directly (no per-head transposes)
            bv = ln_pool.tile([128, D], F32, tag="bv", name="bv")
            nc.sync.dma_start(bv[:], bvrep_d.ap()[l])
            vTt = [attn_pool.tile([128, D], BF16, tag=f"vT{j}", name=f"vT{j}")
                   for j in range(2)]
            for m in range(2):
                ps = psum_mm.tile([128, D], F32, tag="mmp", name="mmp")
                for k in range(2):
                    nc.tensor.matmul(ps[:], yT[k][:, m * 128:(m + 1) * 128],
                                     wq[k][:, 512:768],
                                     start=(k == 0), stop=(k == 1))
                nc.vector.tensor_tensor(vTt[m][:], ps[:], bv[:], AT.add)

            # ---- attention (transposed-score, deferred-normalization) ----
            # scores are tiny (no max-subtraction needed); compute
            # sT[tk,tq] directly so exp(sT) feeds the o-matmul without a
            # per-head transpose, and divide by the softmax denominators
            # once at the end via a PE row-broadcast.
            o_t = [attn_pool.tile([128, 256], BF16, tag=f"o{j}", name=f"o{j}") for j in range(2)]
            den_ps = psum_den.tile([8, 256], F32, tag="den", name="den")
            for h in range(NH):
                r0 = 32 * (h % 2)
                q_h = qkv[h // 2][r0:r0 + HD, :]
                k_h = qkv[4 + h // 2][r0:r0 + HD, :]
                eT = [attn_pool.tile([128, 256], BF16, tag=f"att{j}", name=f"att{j}")
                      for j in range(2)]
                for m in range(2):  # tk chunk
                    ps = psum_sc.tile([128, 256], F32, tag="scp", name="scp")
                    nc.tensor.matmul(ps[:], k_h[:, m * 128:(m + 1) * 128], q_h,
                                     start=True, stop=True)
                    nc.scalar.activation(eT[m][:], ps[:], AF.Exp, scale=scale)
                    # accumulate den_h into row h (other rows get +0)
                    nc.tensor.matmul(den_ps[:], hones[h][:], eT[m][:],
                                     start=(h == 0 and m == 0),
                                     stop=(h == NH - 1 and m == 1))
                # o_feat[dd_h, tq] = sum_tk vT[tk, dd_h] * eT[tk, tq]
                ps = psum_mm.tile([32, 256], F32, tag="mmp", name="mmp")
                for k in range(2):  # tk chunk
                    nc.tensor.matmul(ps[:], vTt[k][:, h * HD:(h + 1) * HD],
                                     eT[k][:], start=(k == 0), stop=(k == 1))
                r1 = (h * HD) % 128
                nc.vector.tensor_copy(o_t[h // 4][r1:r1 + HD, :], ps[:])
            recip8 = attn_pool.tile([8, 256], BF16, tag="rcp8", name="rcp8")
            with nc.allow_low_precision(reason="softmax denom bcast in bf16"):
                nc.vector.reciprocal(recip8[:], den_ps[:])
            for j in range(2):
                idp = psum_sc.tile([128, 256], F32, tag="scp", name="scp")
                nc.tensor.matmul(idp[:], expt[j][:], recip8[:],
                                 start=True, stop=True)
                idc = attn_pool.tile([128, 256], BF16, tag=f"idc{j}",
                                     name=f"idc{j}")
                nc.vector.tensor_copy(idc[:], idp[:])
                nc.vector.tensor_tensor(o_t[j][:], o_t[j][:], idc[:], AT.mult)

            # ---- wo + residual ----
            wo = [wq_pool.tile([128, D], BF16, tag=f"wo{k}", name=f"wo{k}") for k in range(2)]
            for k in range(2):
                nc.sync.dma_start(wo[k][:], woT_d.ap()[l, k])
            ao = [act_pool.tile([128, 256], F32, tag=f"ao{m}", name=f"ao{m}") for m in range(2)]
            for m in range(2):
                ps = psum_mm.tile([128, 256], F32, tag="mmp", name="mmp")
                for k in range(2):
                    nc.tensor.matmul(ps[:], wo[k][:, m * 128:(m + 1) * 128],
                                     o_t[k][:], start=(k == 0), stop=(k == 1))
                bc = _bcol(l, "bo", m)
                nc.vector.tensor_scalar(ao[m][:], ps[:],
                                        biases[:, bc:bc + 1], None, AT.add)
            aoT = _transpose_256(nc, psum_tr, act_pool, ao, ident, tag="aoT")
            for ch in range(2):
                nc.vector.tensor_tensor(z[ch][:], z[ch][:], aoT[ch][:], AT.add)

            # ---- MLP ----
            y2 = _layernorm(nc, sc_pool, act_pool, z)
            y2T = _transpose_256(nc, psum_tr, act_pool, y2, ident, tag="y2T",
                                 dt=BF16)
            w1 = [wq_pool.tile([128, 4 * D], BF16, tag=f"w1_{k}", name=f"w1_{k}") for k in range(2)]
            for k in range(2):
                nc.sync.dma_start(w1[k][:], w1T_d.ap()[l, k])
            h1 = [act_pool.tile([128, 256], BF16, tag=f"h1_{m}", name=f"h1_{m}") for m in range(8)]
            for m in range(8):
                ps = psum_mm.tile([128, 256], F32, tag="mmp", name="mmp")
                for k in range(2):
                    nc.tensor.matmul(ps[:], w1[k][:, m * 128:(m + 1) * 128],
                                     y2T[k][:], start=(k == 0), stop=(k == 1))
                bc = _bcol(l, "b1", m)
                nc.scalar.activation(h1[m][:], ps[:], AF.Gelu,
                                     bias=biases[:, bc:bc + 1])
            w2 = [wq_pool.tile([128, D], BF16, tag=f"w2_{k}", name=f"w2_{k}") for k in range(8)]
            for k in range(8):
                nc.sync.dma_start(w2[k][:], w2T_d.ap()[l, k])
            mo = [act_pool.tile([128, 256], F32, tag=f"mo{m}", name=f"mo{m}") for m in range(2)]
            for m in range(2):
                ps = psum_mm.tile([128, 256], F32, tag="mmp", name="mmp")
                for k in range(8):
                    nc.tensor.matmul(ps[:], w2[k][:, m * 128:(m + 1) * 128],
                                     h1[k][:], start=(k == 0), stop=(k == 7))
                bc = _bcol(l, "b2", m)
                nc.vector.tensor_scalar(mo[m][:], ps[:],
                                        biases[:, bc:bc + 1], None, AT.add)
            moT = _transpose_256(nc, psum_tr, act_pool, mo, ident, tag="moT")
            for ch in range(2):
                nc.vector.tensor_tensor(z[ch][:], z[ch][:], moT[ch][:], AT.add)

        # ---- final projection + unpatch ----
        zT = _transpose_256(nc, psum_tr, act_pool, z, ident, tag="zT", dt=BF16)
        pj = [wq_pool.tile([128, PS * PS], BF16, tag=f"pj{k}", name=f"pj{k}") for k in range(2)]
        for k in range(2):
            nc.sync.dma_start(pj[k][:], projT_d.ap()[k])
        pixF = [act_pool.tile([128, 256], F32, tag=f"pixF{m}", name=f"pixF{m}") for m in range(2)]
        for m in range(2):  # ce chunk
            ps = psum_mm.tile([128, 256], F32, tag="mmp", name="mmp")
            for k in range(2):
                nc.tensor.matmul(ps[:], pj[k][:, m * 128:(m + 1) * 128],
                                 zT[k][:], start=(k == 0), stop=(k == 1))
            nc.vector.tensor_scalar(pixF[m][:], ps[:],
                                    biases[:, BCOL_PROJ + m:BCOL_PROJ + m + 1],
                                    None, AT.add)
        pixAD = _transpose_256(nc, psum_tr, act_pool, pixF, ident, tag="pixAD")
        img_re = imgbuf[:].rearrange("(a c d e) -> c a d e", a=16, c=16, d=16,
                                     e=16)
        for ch in range(2):
            for c in range(16):
                nc.sync.dma_start(img_re[c, ch * 8:(ch + 1) * 8],
                                  pixAD[ch][:, c * 16:(c + 1) * 16])

        # ---- conv 3x3 + conv_b + bp residual ----
        # y-shifts via PE matmuls with host-shipped shift matrices
        # (zero rows at image edges give zero-padding for free).
        img2d = imgbuf[:].rearrange("(y x) -> y x", y=256)
        bp2ds = [q[:].rearrange("(y x) -> y x", y=64) for q in rs_outq]
        with (
            tc.tile_pool(name="cin", bufs=1) as cin_pool,
            tc.tile_pool(name="cout", bufs=1) as cout_pool,
        ):
            conv_psum = psum_sc
            shm = cin_pool.tile([128, 4 * 128], F32, name="shm")
            nc.sync.dma_start(shm[:], shifts_d.ap())
            S_P, S_M, B_P, B_M = (shm[:, i * 128:(i + 1) * 128] for i in range(4))
            ci = [cin_pool.tile([128, 256], F32, tag=f"ci{c}", name=f"ci{c}") for c in range(2)]
            bp_t = [cin_pool.tile([128, 256], F32, tag=f"cb{c}", name=f"cb{c}") for c in range(2)]
            for c in range(2):
                nc.sync.dma_start(ci[c][:], img2d[c * 128:(c + 1) * 128])
                nc.sync.dma_start(bp_t[c][0:64, :], bp2ds[2 * c])
                nc.sync.dma_start(bp_t[c][64:128, :], bp2ds[2 * c + 1])
            co = [cout_pool.tile([128, 256], F32, tag=f"co{c}", name=f"co{c}") for c in range(2)]
            for c in range(2):
                nc.vector.memset(co[c][:], 0.0)

            def wcol(dy, dx):
                col = BCOL_CONVW + (dy + 1) * 3 + (dx + 1)
                return biases[:, col:col + 1]

            for c in range(2):
                # dy=+1 source: in[y+1]
                ps_p = conv_psum.tile([128, 256], F32, tag="scp", name="scp")
                nc.tensor.matmul(ps_p[:], S_P, ci[c][:],
                                 start=True, stop=(c == 1))
                if c == 0:
                    nc.tensor.matmul(ps_p[:], B_P, ci[1][:],
                                     start=False, stop=True)
                # dy=-1 source: in[y-1]
                ps_m = conv_psum.tile([128, 256], F32, tag="scp", name="scp")
                nc.tensor.matmul(ps_m[:], S_M, ci[c][:],
                                 start=True, stop=(c == 0))
                if c == 1:
                    nc.tensor.matmul(ps_m[:], B_M, ci[0][:],
                                     start=False, stop=True)
                for dy, src in ((-1, ps_m), (0, ci[c]), (1, ps_p)):
                    for dx in (-1, 0, 1):
                        xlo, xhi = max(0, dx), 256 + min(0, dx)     # in cols
                        olo, ohi = max(0, -dx), 256 + min(0, -dx)   # out cols
                        nc.vector.scalar_tensor_tensor(
                            co[c][:, olo:ohi], src[:, xlo:xhi],
                            wcol(dy, dx), co[c][:, olo:ohi],
                            op0=AT.mult, op1=AT.add)
            for c in range(2):
                nc.vector.tensor_scalar(co[c][:], co[c][:],
                                        biases[:, BCOL_CONVB:BCOL_CONVB + 1],
                                        None, AT.add)
                nc.vector.tensor_tensor(co[c][:], co[c][:], bp_t[c][:], AT.add)
                nc.sync.dma_start(
                    img_o.ap().rearrange("(y x) -> y x", y=256)[c * 128:(c + 1) * 128],
                    co[c][:])


# ======================= host side =======================

def _host_prep(inputs):
    sino = np.asarray(inputs["sino"], np.float32)      # [B,1,NDET,NT]
    lut = np.asarray(inputs["lut"], np.float32)        # [NY,NX,NDET,2]
    S = sino[:, 0]                                     # [B,NDET,NT]

    kf = lut[..., 0].astype(np.int32)                  # trunc toward 0
    alpha = lut[..., 1].astype(np.float64)
    valid = (kf >= 0) & (kf < NT - 1)
    k0 = np.clip(kf, 0, NT - 2).astype(np.int16)       # [NY,NX,NDET]

    apod = 0.5 - 0.5 * np.cos(2.0 * np.pi * np.arange(NDET) / (NDET - 1))
    apod = apod.astype(np.float32).astype(np.float64)
    denom = float(max(np.float32(apod.sum()), np.float32(1e-6)))
    w0 = (apod[None, None, :] * (1.0 - alpha) * valid / denom).astype(np.float32)
    w1 = (apod[None, None, :] * alpha * valid / denom).astype(np.float32)

    k0_dp = np.ascontiguousarray(k0.reshape(NPIX, NDET).T)     # [NDET, NPIX]
    w0_dp = np.ascontiguousarray(w0.reshape(NPIX, NDET).T)
    w1_dp = np.ascontiguousarray(w1.reshape(NPIX, NDET).T)

    from ml_dtypes import bfloat16
    # LN affine folding: W @ (g*xn + b) + bias == (W*g) @ xn + (bias + W@b)
    ln1_g = np.asarray(inputs["ln1_g"], np.float32)
    ln1_b = np.asarray(inputs["ln1_b"], np.float32)
    ln2_g = np.asarray(inputs["ln2_g"], np.float32)
    ln2_b = np.asarray(inputs["ln2_b"], np.float32)
    wqkv_f = np.asarray(inputs["wqkv"], np.float32)
    w1_f = np.asarray(inputs["w1"], np.float32)
    bqkv_eff = (np.asarray(inputs["bqkv"], np.float32)
                + np.einsum('lod,ld->lo', wqkv_f, ln1_b))
    b1_eff = (np.asarray(inputs["b1"], np.float32)
              + np.einsum('lod,ld->lo', w1_f, ln2_b))
    wqkvT = np.ascontiguousarray(
        wqkv_f.transpose(0, 2, 1) * ln1_g[:, :, None]
    ).reshape(L, 2, 128, 3 * D).astype(bfloat16)
    woT = np.ascontiguousarray(
        np.asarray(inputs["wo"], np.float32).transpose(0, 2, 1)
    ).reshape(L, 2, 128, D).astype(bfloat16)
    w1T = np.ascontiguousarray(
        w1_f.transpose(0, 2, 1) * ln2_g[:, :, None]
    ).reshape(L, 2, 128, 4 * D).astype(bfloat16)
    w2T = np.ascontiguousarray(
        np.asarray(inputs["w2"], np.float32).transpose(0, 2, 1)
    ).reshape(L, 8, 128, D).astype(bfloat16)
    pwT = np.ascontiguousarray(
        np.asarray(inputs["patch_w"], np.float32).T).reshape(2, 128, D).astype(bfloat16)
    projT = np.ascontiguousarray(
        np.asarray(inputs["proj_w"], np.float32).T).reshape(2, 128, PS * PS).astype(bfloat16)
    posb = (np.asarray(inputs["pos_embed"], np.float32)[0]
            + np.asarray(inputs["patch_b"], np.float32)[None, :]
            ).reshape(2, 128, D).copy()
    lnrep = np.empty((L, 4, 128, D), np.float32)
    for l in range(L):
        for j, nm in enumerate(["ln1_g", "ln1_b", "ln2_g", "ln2_b"]):
            lnrep[l, j] = np.asarray(inputs[nm], np.float32)[l][None, :]
    biases = np.zeros((128, NBCOL), np.float32)
    for l in range(L):
        bq = bqkv_eff[l]
        for m in range(12):
            biases[0:64, _bcol(l, "qkv", m)] = bq[m * 64:(m + 1) * 64]
        bo = np.asarray(inputs["bo"], np.float32)[l]
        for m in range(2):
            biases[:, _bcol(l, "bo", m)] = bo[m * 128:(m + 1) * 128]
        b1v = b1_eff[l]
        for m in range(8):
            biases[:, _bcol(l, "b1", m)] = b1v[m * 128:(m + 1) * 128]
        b2v = np.asarray(inputs["b2"], np.float32)[l]
        for m in range(2):
            biases[:, _bcol(l, "b2", m)] = b2v[m * 128:(m + 1) * 128]
    pb = np.asarray(inputs["proj_b"], np.float32)
    for m in range(2):
        biases[:, BCOL_PROJ + m] = pb[m * 128:(m + 1) * 128]
    cw = np.asarray(inputs["conv_w"], np.float32).reshape(9)
    for j in range(9):
        biases[:, BCOL_CONVW + j] = cw[j]
    biases[:, BCOL_CONVB] = np.asarray(inputs["conv_b"], np.float32)[0]

    sel = np.zeros((128, 4), np.float32)
    for k in range(8):
        for j in range(2):
            for b in range(B):
                sel[16 * k + 4 * j + b, b] = 1.0

    bvrep = np.empty((L, 128, D), np.float32)
    for l in range(L):
        bvrep[l] = bqkv_eff[l][512:768][None, :]

    # conv y-shift matrices: psum[m] = sum_k M[k, m] * in[k]
    shifts = np.zeros((128, 4 * 128), np.float32)
    for m in range(127):
        shifts[m + 1, 0 * 128 + m] = 1.0      # S_plus: out[m] = in[m+1]
    for m in range(1, 128):
        shifts[m - 1, 1 * 128 + m] = 1.0      # S_minus: out[m] = in[m-1]
    shifts[0, 2 * 128 + 127] = 1.0            # B_plus: out[127] = next[0]
    shifts[127, 3 * 128 + 0] = 1.0            # B_minus: out[0] = prev[127]

    # expand2[j][p, r] routes head (4j + r//32)'s softmax denominator to
    # o_t[j] partition row r
    expand2 = np.zeros((2, 8, 128), bfloat16)
    for j in range(2):
        for r in range(128):
            expand2[j, 4 * j + r // 32, r] = 1.0
    # hones[h][:, m] = 1 iff m == h: den-matmul lhsT per head
    hones = np.zeros((NH, 128, NH), bfloat16)
    hones[np.arange(NH), :, np.arange(NH)] = 1.0

    shared = dict(wqkvT=wqkvT, woT=woT, w1T=w1T, w2T=w2T, pwT=pwT,
                  projT=projT, posb=posb, lnrep=lnrep, biases=biases, sel=sel,
                  shifts=shifts, bvrep=bvrep,
                  identm=np.eye(128, dtype=np.float32),
                  identb=np.eye(128, dtype=bfloat16),
                  hones=hones, expand2=expand2)

    in_maps = []
    for i in range(N_CORES):
        dets = np.arange(16 * i, 16 * i + 16)
        tbl = np.zeros((128, N_PASS * NT), np.float32)
        idx = np.zeros((N_PASS, N_CHUNK, 128, CHUNK // 16), np.int16)
        wts = np.zeros((N_PASS, N_CHUNK, 128, CHUNK), np.float32)
        for p in range(N_PASS):
            for k in range(8):
                det = int(dets[p * 8 + k])
                for j in range(2):
                    for b in range(B):
                        if j == 0:
                            r = S[b, det]
                        else:
                            r = np.empty(NT, np.float32)
                            r[:NT - 1] = S[b, det, 1:]
                            r[NT - 1] = S[b, det, NT - 1]
                        tbl[16 * k + 4 * j + b, p * NT:(p + 1) * NT] = r
                kd = k0_dp[det].reshape(N_CHUNK, CHUNK // 16, 16)  # [c, s, r]
                idx[p, :, 16 * k:16 * k + 16, :] = kd.transpose(0, 2, 1)
                for j, wsrc in enumerate((w0_dp, w1_dp)):
                    wd = wsrc[det].reshape(N_CHUNK, CHUNK)
                    for b in range(B):
                        wts[p, :, 16 * k + 4 * j + b, :] = wd
        in_maps.append(dict(tbl=tbl, idx=idx, wts=wts, **shared))
    return in_maps


_NC_CACHE = {}


def kernel(**inputs):
    if "nc" not in _NC_CACHE:
        _NC_CACHE["nc"] = build_nc()
    nc = _NC_CACHE["nc"]
    in_maps = _host_prep(inputs)
    res = bass_utils.run_bass_kernel_spmd(
        nc, in_maps, core_ids=list(range(N_CORES)))
    img = np.stack([res.results[b]["img"].reshape(1, NY, NX) for b in range(B)])
    bp = np.stack([res.results[b]["bp"].reshape(1, NY, NX) for b in range(B)])
    return (img, bp)



# revision 66
# speedup vs baseline: 1.0186x; 1.0186x over previous
"""BPTransformer Trainium2 kernel: 8-core SPMD bass implementation.

Sharding: detector-split backprojection (16 dets/NC, all 4 batches,
batch+shift packed on partitions), ReduceScatter (doubled slots) so NC i
receives batch i%4's full backprojected image, then a per-NC transformer
on its batch. Host assembles outputs from cores 0-3.
"""

import numpy as np

import concourse.bass as bass
import concourse.mybir as mybir
import concourse.tile as tile
from concourse import bacc, bass_utils

# problem shapes (hardcoded per contract)
B, NDET, NT, NY, NX = 4, 128, 2048, 256, 256
D, L, NH, PS = 256, 6, 8, 16
HP, WP = NY // PS, NX // PS
N = HP * WP          # 256 tokens
HD = D // NH         # 32
NPIX = NY * NX       # 65536
N_CORES = 8
N_PASS = 2           # dets per q7 core
N_CHUNK = 16
CHUNK = NPIX // N_CHUNK         # 4096
F32 = mybir.dt.float32
BF16 = mybir.dt.bfloat16
I16 = mybir.dt.int16
AT = mybir.AluOpType
AF = mybir.ActivationFunctionType


# bias-column layout in the packed [128, NBCOL] bias tensor
# qkv biases use 12 columns of 64 rows each; the rest use 128 rows.
def _bcol(l, kind, m):
    return l * 24 + {"qkv": 0, "bo": 12, "b1": 14, "b2": 22}[kind] + m


BCOL_PROJ = L * 24          # 144, 2 cols
BCOL_CONVW = BCOL_PROJ + 2  # 146, 9 cols
BCOL_CONVB = BCOL_CONVW + 9  # 155
NBCOL = 156


def build_nc():
    nc = bacc.Bacc("TRN2", target_bir_lowering=False, debug=False,
                   num_devices=N_CORES)

    # ---- per-core inputs ----
    tbl_d = nc.dram_tensor("tbl", [128, N_PASS * NT], F32, kind="ExternalInput")
    idx_d = nc.dram_tensor("idx", [N_PASS, N_CHUNK, 128, CHUNK // 16], I16,
                           kind="ExternalInput")
    wts_d = nc.dram_tensor("wts", [N_PASS, N_CHUNK, 128, CHUNK], F32,
                           kind="ExternalInput")
    sel_d = nc.dram_tensor("sel", [128, 4], F32, kind="ExternalInput")
    # ---- shared transformer weights (matmul operands in bf16) ----
    wqkvT_d = nc.dram_tensor("wqkvT", [L, 2, 128, 3 * D], BF16, kind="ExternalInput")
    woT_d = nc.dram_tensor("woT", [L, 2, 128, D], BF16, kind="ExternalInput")
    w1T_d = nc.dram_tensor("w1T", [L, 2, 128, 4 * D], BF16, kind="ExternalInput")
    w2T_d = nc.dram_tensor("w2T", [L, 8, 128, D], BF16, kind="ExternalInput")
    pwT_d = nc.dram_tensor("pwT", [2, 128, D], BF16, kind="ExternalInput")
    projT_d = nc.dram_tensor("projT", [2, 128, PS * PS], BF16, kind="ExternalInput")
    posb_d = nc.dram_tensor("posb", [2, 128, D], F32, kind="ExternalInput")
    lnrep_d = nc.dram_tensor("lnrep", [L, 4, 128, D], F32, kind="ExternalInput")
    bias_d = nc.dram_tensor("biases", [128, NBCOL], F32, kind="ExternalInput")
    shifts_d = nc.dram_tensor("shifts", [128, 4 * 128], F32, kind="ExternalInput")
    bvrep_d = nc.dram_tensor("bvrep", [L, 128, D], F32, kind="ExternalInput")
    ident_d = nc.dram_tensor("identm", [128, 128], F32, kind="ExternalInput")
    identb_d = nc.dram_tensor("identb", [128, 128], BF16, kind="ExternalInput")
    hones_d = nc.dram_tensor("hones", [NH, 128, NH], BF16, kind="ExternalInput")
    expand_d = nc.dram_tensor("expand2", [2, 8, 128], BF16, kind="ExternalInput")

    # ---- outputs ----
    img_o = nc.dram_tensor("img", [NPIX], F32, kind="ExternalOutput")
    bp_o = nc.dram_tensor("bp", [NPIX], F32, kind="ExternalOutput")

    NQ = 4  # reduce-scatter quarters (first 3 overlap the BP phase)
    with tile.TileContext(nc) as tc:
        with tc.tile_pool(name="dram", bufs=1, space="DRAM") as dram_pool:
            rs_inq = [dram_pool.tile([N_CORES, NPIX // NQ], F32,
                                     name=f"rs_in{q}") for q in range(NQ)]
            rs_outq = [dram_pool.tile([NPIX // NQ], F32, name=f"rs_out{q}")
                       for q in range(NQ)]
            imgbuf = dram_pool.tile([NPIX], F32)

            # ================= backprojection =================
            with (
                tc.tile_pool(name="tbl", bufs=1) as tbl_pool,
                tc.tile_pool(name="selp", bufs=1) as sel_pool,
                tc.tile_pool(name="gidx", bufs=2) as idx_pool,
                tc.tile_pool(name="gout", bufs=2) as g_pool,
                tc.tile_pool(name="gwts", bufs=2) as w_pool,
                tc.tile_pool(name="gacc", bufs=2) as acc_pool,
                tc.tile_pool(name="gtmp", bufs=1) as tmp_pool,
                tc.tile_pool(name="bpsb", bufs=1) as bp_pool,
                tc.tile_pool(name="bpps", bufs=4, space="PSUM") as bp_psum,
            ):
                tbl = tbl_pool.tile([128, N_PASS * NT], F32)
                nc.sync.dma_start(tbl[:], tbl_d.ap())
                sel = sel_pool.tile([128, 4], F32)
                nc.sync.dma_start(sel[:], sel_d.ap())

                for c in range(N_CHUNK):
                    acc = acc_pool.tile([128, CHUNK], F32, tag="acc", name="acc")
                    for p in range(N_PASS):
                        it = idx_pool.tile([128, CHUNK // 16], I16, tag="it", name="it")
                        nc.sync.dma_start(it[:], idx_d.ap()[p, c])
                        wt = w_pool.tile([128, CHUNK], F32, tag="wt", name="wt")
                        nc.sync.dma_start(wt[:], wts_d.ap()[p, c])
                        g = g_pool.tile([128, CHUNK], F32, tag="g", name="g")
                        nc.gpsimd.ap_gather(
                            g[:], tbl[:, p * NT:(p + 1) * NT], it[:],
                            channels=128, num_elems=NT, d=1, num_idxs=CHUNK)
                        if p == 0:
                            nc.vector.tensor_tensor(acc[:], g[:], wt[:], AT.mult)
                        else:
                            tmp = tmp_pool.tile([128, CHUNK], F32, tag="tmp", name="tmp")
                            nc.vector.tensor_tensor(tmp[:], g[:], wt[:], AT.mult)
                            nc.vector.tensor_tensor(acc[:], acc[:], tmp[:], AT.add)
                    # reduce over partitions (dets x shifts) into 4 batch rows
                    bp_sb = bp_pool.tile([4, CHUNK], F32, tag="bpsb", name="bpsb")
                    for blk in range(CHUNK // 512):
                        ps = bp_psum.tile([4, 512], F32, tag="bps", name="bps")
                        nc.tensor.matmul(ps[:], sel[:],
                                         acc[:, blk * 512:(blk + 1) * 512],
                                         start=True, stop=True)
                        nc.scalar.copy(bp_sb[:, blk * 512:(blk + 1) * 512], ps[:])
                    # doubled slots: slot j carries batch j%4
                    cpq = N_CHUNK // NQ
                    rsd = rs_inq[c // cpq]
                    cc = c % cpq
                    nc.sync.dma_start(
                        rsd[0:4, cc * CHUNK:(cc + 1) * CHUNK], bp_sb[:])
                    nc.sync.dma_start(
                        rsd[4:8, cc * CHUNK:(cc + 1) * CHUNK], bp_sb[:])
                    # fire each quarter's reduce-scatter as soon as its
                    # chunks are out, so 3 of 4 overlap remaining BP work
                    if cc == cpq - 1:
                        q = c // cpq
                        nc.gpsimd.collective_compute(
                            "ReduceScatter", AT.add,
                            replica_groups=[list(range(N_CORES))],
                            ins=[rs_inq[q][:].opt()],
                            outs=[rs_outq[q][:].opt()])
                        nc.sync.dma_start(
                            bp_o.ap()[q * (NPIX // NQ):(q + 1) * (NPIX // NQ)],
                            rs_outq[q][:])

            # ================= transformer =================
            _transformer(nc, tc, rs_outq, imgbuf,
                         wqkvT_d, woT_d, w1T_d, w2T_d, pwT_d, projT_d,
                         posb_d, lnrep_d, bias_d, shifts_d, bvrep_d, ident_d,
                         identb_d, hones_d, expand_d, img_o)

    nc.compile()
    return nc


def _transpose_256(nc, psum_tr, dst_pool, src_tiles, ident, tag="tr", dt=F32):
    """src: 2 tiles/APs [128, 256] (partition-chunked 256x256 matrix).
    returns 2 tiles [128, 256]: the transposed matrix, partition-chunked.
    `ident` dtype must match src dtype; dst tiles take dtype `dt`."""
    dst = [dst_pool.tile([128, 256], dt, tag=f"{tag}{j}", name=f"{tag}{j}") for j in range(2)]
    src_dt = src_tiles[0].dtype if hasattr(src_tiles[0], "dtype") else F32
    for ci in range(2):          # src partition chunk
        for fj in range(2):      # src free chunk
            ps = psum_tr.tile([128, 128], src_dt, tag="trp", name="trp")
            nc.tensor.transpose(ps[:], src_tiles[ci][:, fj * 128:(fj + 1) * 128],
                                ident[:])
            nc.vector.tensor_copy(dst[fj][:, ci * 128:(ci + 1) * 128], ps[:])
    return dst


def _layernorm(nc, sc_pool, out_pool, x_tiles):
    """row-layout LN over free dim D (affine gamma/beta folded into the
    downstream matmul weights/biases host-side). x_tiles: 2 x [128, D]."""
    out = []
    for ct in range(2):
        x = x_tiles[ct]
        s = sc_pool.tile([128, 1], F32, tag="s", name="s")
        nc.vector.reduce_sum(s[:], x[:], mybir.AxisListType.X)
        nm = sc_pool.tile([128, 1], F32, tag="nm", name="nm")
        nc.vector.tensor_scalar(nm[:], s[:], -1.0 / D, None, AT.mult)
        xc = out_pool.tile([128, D], F32, tag="xc", name="xc")
        nc.vector.tensor_scalar(xc[:], x[:], nm[:], None, AT.add)
        sq = out_pool.tile([128, D], F32, tag="sq", name="sq")
        nc.vector.tensor_tensor(sq[:], xc[:], xc[:], AT.mult)
        vs = sc_pool.tile([128, 1], F32, tag="vs", name="vs")
        nc.vector.reduce_sum(vs[:], sq[:], mybir.AxisListType.X)
        vs2 = sc_pool.tile([128, 1], F32, tag="vs2", name="vs2")
        nc.vector.tensor_scalar(vs2[:], vs[:], 1.0 / D, 1e-5, AT.mult, AT.add)
        std = sc_pool.tile([128, 1], F32, tag="std", name="std")
        nc.scalar.activation(std[:], vs2[:], AF.Sqrt)
        inv = sc_pool.tile([128, 1], F32, tag="inv", name="inv")
        nc.vector.reciprocal(inv[:], std[:])
        y = out_pool.tile([128, D], F32, tag="lny", name="lny")
        nc.vector.tensor_scalar(y[:], xc[:], inv[:], None, AT.mult)
        out.append(y)
    return out


def _transformer(nc, tc, rs_outq, imgbuf, wqkvT_d, woT_d, w1T_d, w2T_d,
                 pwT_d, projT_d, posb_d, lnrep_d, bias_d, shifts_d, bvrep_d,
                 ident_d, identb_d, hones_d, expand_d, img_o):
    with (
        tc.tile_pool(name="ident", bufs=1) as id_pool,
        tc.tile_pool(name="bias", bufs=1) as bias_pool,
        tc.tile_pool(name="zst", bufs=1) as z_pool,          # persistent z
        tc.tile_pool(name="lnv", bufs=2) as ln_pool,
        tc.tile_pool(name="wq", bufs=2) as wq_pool,
        tc.tile_pool(name="act", bufs=2) as act_pool,        # transient acts
        tc.tile_pool(name="sc", bufs=8) as sc_pool,          # [128,1] scalars
        tc.tile_pool(name="attn", bufs=2) as attn_pool,
        tc.tile_pool(name="pst", bufs=2, space="PSUM") as psum_tr,
        tc.tile_pool(name="psm", bufs=2, space="PSUM") as psum_mm,
        tc.tile_pool(name="pss", bufs=2, space="PSUM") as psum_sc,
        tc.tile_pool(name="psd", bufs=1, space="PSUM") as psum_den,
    ):
        ident = id_pool.tile([128, 128], F32, name="ident")
        nc.sync.dma_start(ident[:], ident_d.ap())
        identb = id_pool.tile([128, 128], BF16, name="identb")
        nc.sync.dma_start(identb[:], identb_d.ap())
        hones = [id_pool.tile([128, NH], BF16, tag=f"ho{h}", name=f"ho{h}")
                 for h in range(NH)]
        for h in range(NH):
            nc.sync.dma_start(hones[h][:], hones_d.ap()[h])
        expt = [id_pool.tile([8, 128], BF16, tag=f"expt{j}", name=f"expt{j}")
                for j in range(2)]
        for j in range(2):
            nc.sync.dma_start(expt[j][:], expand_d.ap()[j])
        biases = bias_pool.tile([128, NBCOL], F32, name="biases")
        nc.sync.dma_start(biases[:], bias_d.ap())

        bp_res = [q[:].rearrange("(a c d e) -> c a d e", a=4, c=16, d=16, e=16)
                  for q in rs_outq]

        # ---- patch extract: X [tok, (c,e)] row layout ----
        X = [act_pool.tile([128, 256], F32, tag=f"X{j}", name=f"X{j}") for j in range(2)]
        for ch in range(2):
            for c in range(16):
                nc.sync.dma_start(X[ch][0:64, c * 16:(c + 1) * 16],
                                  bp_res[2 * ch][c])
                nc.sync.dma_start(X[ch][64:128, c * 16:(c + 1) * 16],
                                  bp_res[2 * ch + 1][c])
        XT = _transpose_256(nc, psum_tr, act_pool, X, ident, tag="XT", dt=BF16)
        # z0 [tok, D] = X @ pw.T : lhsT = XT chunks, rhs = pwT
        pw = [wq_pool.tile([128, D], BF16, tag=f"pw{k}", name=f"pw{k}") for k in range(2)]
        for k in range(2):
            nc.sync.dma_start(pw[k][:], pwT_d.ap()[k])
        posb = [wq_pool.tile([128, D], F32, tag=f"posb{j}", name=f"posb{j}") for j in range(2)]
        for ch in range(2):
            nc.sync.dma_start(posb[ch][:], posb_d.ap()[ch])
        z = [z_pool.tile([128, D], F32, tag=f"z{ch}", name=f"z{ch}") for ch in range(2)]
        for m in range(2):  # token chunk
            ps = psum_mm.tile([128, D], F32, tag="mmp", name="mmp")
            for k in range(2):  # ce chunk
                nc.tensor.matmul(ps[:], XT[k][:, m * 128:(m + 1) * 128],
                                 pw[k][:], start=(k == 0), stop=(k == 1))
            nc.vector.tensor_tensor(z[m][:], ps[:], posb[m][:], AT.add)

        scale = 1.0 / float(np.sqrt(HD))
        for l in range(L):
            # ---- LN1 ----
            y = _layernorm(nc, sc_pool, act_pool, z)
            yT = _transpose_256(nc, psum_tr, act_pool, y, ident, tag="yT",
                                dt=BF16)

            # ---- qkv [feat, tok] ----
            wq = [wq_pool.tile([128, 3 * D], BF16, tag=f"wqkv{k}", name=f"wqkv{k}") for k in range(2)]
            for k in range(2):
                nc.sync.dma_start(wq[k][:], wqkvT_d.ap()[l, k])
            qkv = [act_pool.tile([64, 256], BF16, tag=f"qkv{m}", name=f"qkv{m}") for m in range(8)]
            for m in range(8):
                ps = psum_mm.tile([64, 256], F32, tag="mmp", name="mmp")
                for k in range(2):
                    nc.tensor.matmul(ps[:], wq[k][:, m * 64:(m + 1) * 64],
                                     yT[k][:], start=(k == 0), stop=(k == 1))
                bc = _bcol(l, "qkv", m)
                nc.vector.tensor_scalar(qkv[m][:], ps[:],
                                        biases[0:64, bc:bc + 1], None, AT.add)
            # vT [tok, v-feat] computed directly (no per-head transposes)
            bv = ln_pool.tile([128, D], F32, tag="bv", name="bv")
            nc.sync.dma_start(bv[:], bvrep_d.ap()[l])
            vTt = [attn_pool.tile([128, D], BF16, tag=f"vT{j}", name=f"vT{j}")
                   for j in range(2)]
            for m in range(2):
                ps = psum_mm.tile([128, D], F32, tag="mmp", name="mmp")
                for k in range(2):
                    nc.tensor.matmul(ps[:], yT[k][:, m * 128:(m + 1) * 128],
                                     wq[k][:, 512:768],
                                     start=(k == 0), stop=(k == 1))
                nc.vector.tensor_tensor(vTt[m][:], ps[:], bv[:], AT.add)

            # ---- attention (transposed-score, deferred-normalization) ----
            # scores are tiny (no max-subtraction needed); compute
            # sT[tk,tq] directly so exp(sT) feeds the o-matmul without a
            # per-head transpose, and divide by the softmax denominators
            # once at the end via a PE row-broadcast.
            o_t = [attn_pool.tile([128, 256], BF16, tag=f"o{j}", name=f"o{j}") for j in range(2)]
            den_ps = psum_den.tile([8, 256], F32, tag="den", name="den")
            for h in range(NH):
                r0 = 32 * (h % 2)
                q_h = qkv[h // 2][r0:r0 + HD, :]
                k_h = qkv[4 + h // 2][r0:r0 + HD, :]
                eT = [attn_pool.tile([128, 256], BF16, tag=f"att{j}", name=f"att{j}")
                      for j in range(2)]
                for m in range(2):  # tk chunk
                    ps = psum_sc.tile([128, 256], F32, tag="scp", name="scp")
                    nc.tensor.matmul(ps[:], k_h[:, m * 128:(m + 1) * 128], q_h,
                                     start=True, stop=True)
                    nc.scalar.activation(eT[m][:], ps[:], AF.Exp, scale=scale)
                    # accumulate den_h into row h (other rows get +0)
                    nc.tensor.matmul(den_ps[:], hones[h][:], eT[m][:],
                                     start=(h == 0 and m == 0),
                                     stop=(h == NH - 1 and m == 1))
                # o_feat[dd_h, tq] = sum_tk vT[tk, dd_h] * eT[tk, tq]
                ps = psum_mm.tile([32, 256], F32, tag="mmp", name="mmp")
                for k in range(2):  # tk chunk
                    nc.tensor.matmul(ps[:], vTt[k][:, h * HD:(h + 1) * HD],
                                     eT[k][:], start=(k == 0), stop=(k == 1))
                r1 = (h * HD) % 128
                nc.vector.tensor_copy(o_t[h // 4][r1:r1 + HD, :], ps[:])
            recip8 = attn_pool.tile([8, 256], BF16, tag="rcp8", name="rcp8")
            with nc.allow_low_precision(reason="softmax denom bcast in bf16"):
                nc.vector.reciprocal(recip8[:], den_ps[:])
            for j in range(2):
                idp = psum_sc.tile([128, 256], F32, tag="scp", name="scp")
                nc.tensor.matmul(idp[:], expt[j][:], recip8[:],
                                 start=True, stop=True)
                idc = attn_pool.tile([128, 256], BF16, tag=f"idc{j}",
                                     name=f"idc{j}")
                nc.vector.tensor_copy(idc[:], idp[:])
                nc.vector.tensor_tensor(o_t[j][:], o_t[j][:], idc[:], AT.mult)

            # ---- wo + residual ----
            wo = [wq_pool.tile([128, D], BF16, tag=f"wo{k}", name=f"wo{k}") for k in range(2)]
            for k in range(2):
                nc.sync.dma_start(wo[k][:], woT_d.ap()[l, k])
            ao = [act_pool.tile([128, 256], F32, tag=f"ao{m}", name=f"ao{m}") for m in range(2)]
            for m in range(2):
                ps = psum_mm.tile([128, 256], F32, tag="mmp", name="mmp")
                for k in range(2):
                    nc.tensor.matmul(ps[:], wo[k][:, m * 128:(m + 1) * 128],
                                     o_t[k][:], start=(k == 0), stop=(k == 1))
                bc = _bcol(l, "bo", m)
                nc.vector.tensor_scalar(ao[m][:], ps[:],
                                        biases[:, bc:bc + 1], None, AT.add)
            aoT = _transpose_256(nc, psum_tr, act_pool, ao, ident, tag="aoT")
            for ch in range(2):
                nc.vector.tensor_tensor(z[ch][:], z[ch][:], aoT[ch][:], AT.add)

            # ---- MLP ----
            y2 = _layernorm(nc, sc_pool, act_pool, z)
            y2T = _transpose_256(nc, psum_tr, act_pool, y2, ident, tag="y2T",
                                 dt=BF16)
            w1 = [wq_pool.tile([128, 4 * D], BF16, tag=f"w1_{k}", name=f"w1_{k}") for k in range(2)]
            for k in range(2):
                nc.sync.dma_start(w1[k][:], w1T_d.ap()[l, k])
            h1 = [act_pool.tile([128, 256], BF16, tag=f"h1_{m}", name=f"h1_{m}") for m in range(8)]
            for m in range(8):
                ps = psum_mm.tile([128, 256], F32, tag="mmp", name="mmp")
                for k in range(2):
                    nc.tensor.matmul(ps[:], w1[k][:, m * 128:(m + 1) * 128],
                                     y2T[k][:], start=(k == 0), stop=(k == 1))
                bc = _bcol(l, "b1", m)
                nc.scalar.activation(h1[m][:], ps[:], AF.Gelu,
                                     bias=biases[:, bc:bc + 1])
            w2 = [wq_pool.tile([128, D], BF16, tag=f"w2_{k}", name=f"w2_{k}") for k in range(8)]
            for k in range(8):
                nc.sync.dma_start(w2[k][:], w2T_d.ap()[l, k])
            mo = [act_pool.tile([128, 256], F32, tag=f"mo{m}", name=f"mo{m}") for m in range(2)]
            for m in range(2):
                ps = psum_mm.tile([128, 256], F32, tag="mmp", name="mmp")
                for k in range(8):
                    nc.tensor.matmul(ps[:], w2[k][:, m * 128:(m + 1) * 128],
                                     h1[k][:], start=(k == 0), stop=(k == 7))
                bc = _bcol(l, "b2", m)
                nc.vector.tensor_scalar(mo[m][:], ps[:],
                                        biases[:, bc:bc + 1], None, AT.add)
            moT = _transpose_256(nc, psum_tr, act_pool, mo, ident, tag="moT")
            for ch in range(2):
                nc.vector.tensor_tensor(z[ch][:], z[ch][:], moT[ch][:], AT.add)

        # ---- final projection + unpatch ----
        zT = _transpose_256(nc, psum_tr, act_pool, z, ident, tag="zT", dt=BF16)
        pj = [wq_pool.tile([128, PS * PS], BF16, tag=f"pj{k}", name=f"pj{k}") for k in range(2)]
        for k in range(2):
            nc.sync.dma_start(pj[k][:], projT_d.ap()[k])
        pixF = [act_pool.tile([128, 256], F32, tag=f"pixF{m}", name=f"pixF{m}") for m in range(2)]
        for m in range(2):  # ce chunk
            ps = psum_mm.tile([128, 256], F32, tag="mmp", name="mmp")
            for k in range(2):
                nc.tensor.matmul(ps[:], pj[k][:, m * 128:(m + 1) * 128],
                                 zT[k][:], start=(k == 0), stop=(k == 1))
            nc.vector.tensor_scalar(pixF[m][:], ps[:],
                                    biases[:, BCOL_PROJ + m:BCOL_PROJ + m + 1],
                                    None, AT.add)
        pixAD = _transpose_256(nc, psum_tr, act_pool, pixF, ident, tag="pixAD")
        img_re = imgbuf[:].rearrange("(a c d e) -> c a d e", a=16, c=16, d=16,
                                     e=16)
        for ch in range(2):
            for c in range(16):
                nc.sync.dma_start(img_re[c, ch * 8:(ch + 1) * 8],
                                  pixAD[ch][:, c * 16:(c + 1) * 16])

        # ---- conv 3x3 + conv_b + bp residual ----
        # y-shifts via PE matmuls with host-shipped shift matrices
        # (zero rows at image edges give zero-padding for free).
        img2d = imgbuf[:].rearrange("(y x) -> y x", y=256)
        bp2ds = [q[:].rearrange("(y x) -> y x", y=64) for q in rs_outq]
        with (
            tc.tile_pool(name="cin", bufs=1) as cin_pool,
            tc.tile_pool(name="cout", bufs=1) as cout_pool,
        ):
            conv_psum = psum_sc
            shm = cin_pool.tile([128, 4 * 128], F32, name="shm")
            nc.sync.dma_start(shm[:], shifts_d.ap())
            S_P, S_M, B_P, B_M = (shm[:, i * 128:(i + 1) * 128] for i in range(4))
            ci = [cin_pool.tile([128, 256], F32, tag=f"ci{c}", name=f"ci{c}") for c in range(2)]
            bp_t = [cin_pool.tile([128, 256], F32, tag=f"cb{c}", name=f"cb{c}") for c in range(2)]
            for c in range(2):
                nc.sync.dma_start(ci[c][:], img2d[c * 128:(c + 1) * 128])
                nc.sync.dma_start(bp_t[c][0:64, :], bp2ds[2 * c])
                nc.sync.dma_start(bp_t[c][64:128, :], bp2ds[2 * c + 1])
            co = [cout_pool.tile([128, 256], F32, tag=f"co{c}", name=f"co{c}") for c in range(2)]
            for c in range(2):
                nc.vector.memset(co[c][:], 0.0)

            def wcol(dy, dx):
                col = BCOL_CONVW + (dy + 1) * 3 + (dx + 1)
                return biases[:, col:col + 1]

            for c in range(2):
                # dy=+1 source: in[y+1]
                ps_p = conv_psum.tile([128, 256], F32, tag="scp", name="scp")
                nc.tensor.matmul(ps_p[:], S_P, ci[c][:],
                                 start=True, stop=(c == 1))
                if c == 0:
                    nc.tensor.matmul(ps_p[:], B_P, ci[1][:],
                                     start=False, stop=True)
                # dy=-1 source: in[y-1]
                ps_m = conv_psum.tile([128, 256], F32, tag="scp", name="scp")
                nc.tensor.matmul(ps_m[:], S_M, ci[c][:],
                                 start=True, stop=(c == 0))
                if c == 1:
                    nc.tensor.matmul(ps_m[:], B_M, ci[0][:],
                                     start=False, stop=True)
                for dy, src in ((-1, ps_m), (0, ci[c]), (1, ps_p)):
                    for dx in (-1, 0, 1):
                        xlo, xhi = max(0, dx), 256 + min(0, dx)     # in cols
                        olo, ohi = max(0, -dx), 256 + min(0, -dx)   # out cols
                        nc.vector.scalar_tensor_tensor(
                            co[c][:, olo:ohi], src[:, xlo:xhi],
                            wcol(dy, dx), co[c][:, olo:ohi],
                            op0=AT.mult, op1=AT.add)
            for c in range(2):
                nc.vector.tensor_scalar(co[c][:], co[c][:],
                                        biases[:, BCOL_CONVB:BCOL_CONVB + 1],
                                        None, AT.add)
                nc.vector.tensor_tensor(co[c][:], co[c][:], bp_t[c][:], AT.add)
                nc.sync.dma_start(
                    img_o.ap().rearrange("(y x) -> y x", y=256)[c * 128:(c + 1) * 128],
                    co[c][:])


# ======================= host side =======================

def _host_prep(inputs):
    sino = np.asarray(inputs["sino"], np.float32)      # [B,1,NDET,NT]
    lut = np.asarray(inputs["lut"], np.float32)        # [NY,NX,NDET,2]
    S = sino[:, 0]                                     # [B,NDET,NT]

    kf = lut[..., 0].astype(np.int32)                  # trunc toward 0
    alpha = lut[..., 1].astype(np.float64)
    valid = (kf >= 0) & (kf < NT - 1)
    k0 = np.clip(kf, 0, NT - 2).astype(np.int16)       # [NY,NX,NDET]

    apod = 0.5 - 0.5 * np.cos(2.0 * np.pi * np.arange(NDET) / (NDET - 1))
    apod = apod.astype(np.float32).astype(np.float64)
    denom = float(max(np.float32(apod.sum()), np.float32(1e-6)))
    w0 = (apod[None, None, :] * (1.0 - alpha) * valid / denom).astype(np.float32)
    w1 = (apod[None, None, :] * alpha * valid / denom).astype(np.float32)

    k0_dp = np.ascontiguousarray(k0.reshape(NPIX, NDET).T)     # [NDET, NPIX]
    w0_dp = np.ascontiguousarray(w0.reshape(NPIX, NDET).T)
    w1_dp = np.ascontiguousarray(w1.reshape(NPIX, NDET).T)

    from ml_dtypes import bfloat16
    # LN affine folding: W @ (g*xn + b) + bias == (W*g) @ xn + (bias + W@b)
    ln1_g = np.asarray(inputs["ln1_g"], np.float32)
    ln1_b = np.asarray(inputs["ln1_b"], np.float32)
    ln2_g = np.asarray(inputs["ln2_g"], np.float32)
    ln2_b = np.asarray(inputs["ln2_b"], np.float32)
    wqkv_f = np.asarray(inputs["wqkv"], np.float32)
    w1_f = np.asarray(inputs["w1"], np.float32)
    bqkv_eff = (np.asarray(inputs["bqkv"], np.float32)
                + np.einsum('lod,ld->lo', wqkv_f, ln1_b))
    b1_eff = (np.asarray(inputs["b1"], np.float32)
              + np.einsum('lod,ld->lo', w1_f, ln2_b))
    wqkvT = np.ascontiguousarray(
        wqkv_f.transpose(0, 2, 1) * ln1_g[:, :, None]
    ).reshape(L, 2, 128, 3 * D).astype(bfloat16)
    woT = np.ascontiguousarray(
        np.asarray(inputs["wo"], np.float32).transpose(0, 2, 1)
    ).reshape(L, 2, 128, D).astype(bfloat16)
    w1T = np.ascontiguousarray(
        w1_f.transpose(0, 2, 1) * ln2_g[:, :, None]
    ).reshape(L, 2, 128, 4 * D).astype(bfloat16)
    w2T = np.ascontiguousarray(
        np.asarray(inputs["w2"], np.float32).transpose(0, 2, 1)
    ).reshape(L, 8, 128, D).astype(bfloat16)
    pwT = np.ascontiguousarray(
        np.asarray(inputs["patch_w"], np.float32).T).reshape(2, 128, D).astype(bfloat16)
    projT = np.ascontiguousarray(
        np.asarray(inputs["proj_w"], np.float32).T).reshape(2, 128, PS * PS).astype(bfloat16)
    posb = (np.asarray(inputs["pos_embed"], np.float32)[0]
            + np.asarray(inputs["patch_b"], np.float32)[None, :]
            ).reshape(2, 128, D).copy()
    lnrep = np.empty((L, 4, 128, D), np.float32)
    for l in range(L):
        for j, nm in enumerate(["ln1_g", "ln1_b", "ln2_g", "ln2_b"]):
            lnrep[l, j] = np.asarray(inputs[nm], np.float32)[l][None, :]
    biases = np.zeros((128, NBCOL), np.float32)
    for l in range(L):
        bq = bqkv_eff[l]
        for m in range(12):
            biases[0:64, _bcol(l, "qkv", m)] = bq[m * 64:(m + 1) * 64]
        bo = np.asarray(inputs["bo"], np.float32)[l]
        for m in range(2):
            biases[:, _bcol(l, "bo", m)] = bo[m * 128:(m + 1) * 128]
        b1v = b1_eff[l]
        for m in range(8):
            biases[:, _bcol(l, "b1", m)] = b1v[m * 128:(m + 1) * 128]
        b2v = np.asarray(inputs["b2"], np.float32)[l]
        for m in range(2):
            biases[:, _bcol(l, "b2", m)] = b2v[m * 128:(m + 1) * 128]
    pb = np.asarray(inputs["proj_b"], np.float32)
    for m in range(2):
        biases[:, BCOL_PROJ + m] = pb[m * 128:(m + 1) * 128]
    cw = np.asarray(inputs["conv_w"], np.float32).reshape(9)
    for j in range(9):
        biases[:, BCOL_CONVW + j] = cw[j]
    biases[:, BCOL_CONVB] = np.asarray(inputs["conv_b"], np.float32)[0]

    sel = np.zeros((128, 4), np.float32)
    for k in range(8):
        for j in range(2):
            for b in range(B):
                sel[16 * k + 4 * j + b, b] = 1.0

    bvrep = np.empty((L, 128, D), np.float32)
    for l in range(L):
        bvrep[l] = bqkv_eff[l][512:768][None, :]

    # conv y-shift matrices: psum[m] = sum_k M[k, m] * in[k]
    shifts = np.zeros((128, 4 * 128), np.float32)
    for m in range(127):
        shifts[m + 1, 0 * 128 + m] = 1.0      # S_plus: out[m] = in[m+1]
    for m in range(1, 128):
        shifts[m - 1, 1 * 128 + m] = 1.0      # S_minus: out[m] = in[m-1]
    shifts[0, 2 * 128 + 127] = 1.0            # B_plus: out[127] = next[0]
    shifts[127, 3 * 128 + 0] = 1.0            # B_minus: out[0] = prev[127]

    # expand2[j][p, r] routes head (4j + r//32)'s softmax denominator to
    # o_t[j] partition row r
    expand2 = np.zeros((2, 8, 128), bfloat16)
    for j in range(2):
        for r in range(128):
            expand2[j, 4 * j + r // 32, r] = 1.0
    # hones[h][:, m] = 1 iff m == h: den-matmul lhsT per head
    hones = np.zeros((NH, 128, NH), bfloat16)
    hones[np.arange(NH), :, np.arange(NH)] = 1.0

    shared = dict(wqkvT=wqkvT, woT=woT, w1T=w1T, w2T=w2T, pwT=pwT,
                  projT=projT, posb=posb, lnrep=lnrep, biases=biases, sel=sel,
                  shifts=shifts, bvrep=bvrep,
                  identm=np.eye(128, dtype=np.float32),
                  identb=np.eye(128, dtype=bfloat16),
                  hones=hones, expand2=expand2)

    in_maps = []
    for i in range(N_CORES):
        dets = np.arange(16 * i, 16 * i + 16)
        tbl = np.zeros((128, N_PASS * NT), np.float32)
        idx = np.zeros((N_PASS, N_CHUNK, 128, CHUNK // 16), np.int16)
        wts = np.zeros((N_PASS, N_CHUNK, 128, CHUNK), np.float32)
        for p in range(N_PASS):
            for k in range(8):
                det = int(dets[p * 8 + k])
                for j in range(2):
                    for b in range(B):
                        if j == 0:
                            r = S[b, det]
                        else:
                            r = np.empty(NT, np.float32)
                            r[:NT - 1] = S[b, det, 1:]
                            r[NT - 1] = S[b, det, NT - 1]
                        tbl[16 * k + 4 * j + b, p * NT:(p + 1) * NT] = r
                kd = k0_dp[det].reshape(N_CHUNK, CHUNK // 16, 16)  # [c, s, r]
                idx[p, :, 16 * k:16 * k + 16, :] = kd.transpose(0, 2, 1)
                for j, wsrc in enumerate((w0_dp, w1_dp)):
                    wd = wsrc[det].reshape(N_CHUNK, CHUNK)
                    for b in range(B):
                        wts[p, :, 16 * k + 4 * j + b, :] = wd
        in_maps.append(dict(tbl=tbl, idx=idx, wts=wts, **shared))
    return in_maps


_NC_CACHE = {}


def kernel(**inputs):
    if "nc" not in _NC_CACHE:
        _NC_CACHE["nc"] = build_nc()
    nc = _NC_CACHE["nc"]
    in_maps = _host_prep(inputs)
    res = bass_utils.run_bass_kernel_spmd(
        nc, in_maps, core_ids=list(range(N_CORES)))
    img = np.stack([res.results[b]["img"].reshape(1, NY, NX) for b in range(B)])
    bp = np.stack([res.results[b]["bp"].reshape(1, NY, NX) for b in range(B)])
    return (img, bp)

